# revision 1
# baseline (speedup 1.0000x reference)
"""BinConv3d (sign-binarized 3x3x3 conv, NCDHW) on 8 Trainium2 NeuronCores.

Full inputs in, full output out. Sharding: depth dim D=64 split 8 ways
(8 output planes per core) with a 1-plane halo on the input; conv weights
and bias replicated to every core.

Host prep: each core's input slab is rearranged to [plane, quarter, ci,
34, 130] fp32 — the H dim split into 4 quarter-row panels of 32 rows,
each padded with 1 halo row above/below and 1 zero col left/right, so
the device needs no data reshuffling at all.

Per-core kernel (Bass/Tile):
  - One [128, 34*130] DMA per plane (partition = quarter*32 + ci), then
    ScalarE Sign fp32 -> bf16 (zero pads stay zero).
  - Conv = 27 accumulating matmuls (K=32 ci, M=64 co, N=512) per 4-row
    output tile; every (kd, kh, kw) tap is a free-dim AP offset.
  - 16-way PE tiling: quarter q runs on PE row-group q (tile_position
    row 32q); even/odd 4-row blocks run on PE column halves. 8 matmuls
    issue back-to-back per tap and run concurrently: full 128x128 array.
  - PSUM: 4 banks per generation (bank = quarter, partitions 0-63 even
    block / 64-127 odd block), double-buffered = all 8 banks.
  - PSUM drained with bias add on ScalarE (even) / VectorE (odd) into a
    [128, 2048] staging tile, flushed to HBM as 2x512KB DMAs on
    complementary SBUF port sets.
"""

import numpy as np
import ml_dtypes

import concourse.bass as bass
import concourse.mybir as mybir
import concourse.tile as tile
from concourse import bacc
from concourse.bass import ts
from concourse.bass_utils import run_bass_kernel_spmd

CI = 32
CO = 64
D_FULL = 64
N_CORES = 8
D_OUT = D_FULL // N_CORES  # output planes per core
D_IN = D_OUT + 2  # input planes per core (1-plane halo each side)

_cache = {}


def build_conv_program(n_in_planes=D_IN, n_out_planes=D_OUT, H=128, W=128,
                       debug=False):
    """Build the per-core Bass program (SPMD: same program on all cores)."""
    f32 = mybir.dt.float32
    bf16 = mybir.dt.bfloat16
    Hq = H // 4          # rows per quarter-panel
    Hqp, Wp = Hq + 2, W + 2
    n_pairs = Hq // 8    # even/odd block pairs per quarter
    assert Hq % 8 == 0 and W == 128

    nc = bacc.Bacc("TRN2", target_bir_lowering=False, debug=debug)
    x_in = nc.declare_dram_parameter(
        "xs", [n_in_planes, 4, CI, Hqp, Wp], f32, isOutput=False)
    w_in = nc.declare_dram_parameter("wst", [128, 27, 2 * CO], bf16,
                                     isOutput=False)
    b_in = nc.declare_dram_parameter("bias", [128, 1], f32, isOutput=False)
    y_out = nc.declare_dram_parameter("y", [CO, n_out_planes, H, W], f32,
                                      isOutput=True)

    with tile.TileContext(nc) as tc:
        with (
            tc.tile_pool(name="const", bufs=1) as constp,
            tc.tile_pool(name="raw", bufs=2) as rawp,
            tc.tile_pool(name="sgn", bufs=5) as sgnp,
            tc.tile_pool(name="stg", bufs=4) as stgp,
            tc.tile_pool(name="psum", bufs=2, space="PSUM") as psump,
        ):
            wt = constp.tile([128, 27, 2 * CO], bf16)
            nc.sync.dma_start(out=wt[:], in_=w_in[:])
            bs = constp.tile([128, 1], f32)
            nc.sync.dma_start(out=bs[:], in_=b_in[:])

            sgns = {}
            # panel rows are loaded in two halves so the first matmuls can
            # start after half of planes 0-2 arrived: top covers panel rows
            # [0, Hh+2) (blocks 0..Hq/8-1), bottom rows [Hh, Hqp)
            Hh = Hq // 2

            def load_half(p, lo, n):
                raw = rawp.tile([128, n, Wp], f32, tag="raw")
                nc.sync.dma_start(
                    out=raw[:],
                    in_=x_in[p, :, :, lo:lo + n].rearrange(
                        "q c h w -> (q c) h w"),
                )
                sgn = sgnp.tile([128, n, Wp], bf16, tag="sgn")
                nc.scalar.sign(sgn[:], raw[:])
                return sgn

            def load_plane(p, part):
                if part == 0:
                    sgns[p] = [load_half(p, 0, Hh + 2), None]
                else:
                    sgns[p][1] = load_half(p, Hh, Hh + 2)

            for p in range(3):
                load_plane(p, 0)
            for p in range(3):
                load_plane(p, 1)

            for d in range(n_out_planes):
                if d + 3 < n_in_planes:
                    load_plane(d + 3, 0)
                    load_plane(d + 3, 1)
                for pi in range(n_pairs):
                    # generation: for each quarter, blocks 2*pi (even,
                    # cols 0-63) and 2*pi+1 (odd, cols 64-127)
                    pts = [psump.tile([128, 512], f32, tag=f"pt{q}",
                                      name=f"pt{q}_{d}_{pi}")
                           for q in range(4)]
                    for tap in range(27):
                        kd, r = divmod(tap, 9)
                        kh, kw = divmod(r, 3)
                        for q in range(4):
                            for half in range(2):
                                blk = 2 * pi + half
                                top = blk < Hq // 8
                                sg = sgns[d + kd][0 if top else 1]
                                row = 4 * blk + kh - (0 if top else Hh)
                                rhs = sg[32 * q:32 * q + 32,
                                         row:row + 4,
                                         kw:kw + W]
                                nc.tensor.matmul(
                                    pts[q][64 * half:64 * half + 64, :],
                                    lhsT=wt[32 * q:32 * q + 32, tap, 0:CO],
                                    rhs=rhs,
                                    start=(tap == 0),
                                    stop=(tap == 26),
                                    tile_position=(32 * q, 64 * half),
                                    skip_group_check=True,
                                )
                    # drain with bias add, split across ScalarE and VectorE
                    stg = stgp.tile([128, 4 * 512], f32, tag="stg")
                    for q in range(4):
                        nc.scalar.activation(
                            stg[0:64, ts(q, 512)], pts[q][0:64, :],
                            mybir.ActivationFunctionType.Identity,
                            bias=bs[0:64], scale=1.0,
                        )
                        nc.vector.tensor_scalar_add(
                            out=stg[64:128, ts(q, 512)], in0=pts[q][64:128, :],
                            scalar1=bs[64:128],
                        )
                    # out rows: quarter q -> 32q + 8*pi + 4*half .. +4
                    yv = y_out[:, d].rearrange("co (q hi) w -> co q hi w", q=4)
                    for half in range(2):
                        dst = yv[:, :, 8 * pi + 4 * half:8 * pi + 4 * half + 4,
                                 :].rearrange("co q hi w -> co q (hi w)")
                        src = stg[64 * half:64 * half + 64, :].rearrange(
                            "co (q n) -> co q n", q=4)
                        nc.sync.dma_start(out=dst, in_=src)

    nc.compile()
    return nc


def _get_program():
    if "nc" not in _cache:
        _cache["nc"] = build_conv_program()
    return _cache["nc"]


def prep_weights(W, b):
    W = np.asarray(W, dtype=np.float32)
    b = np.asarray(b, dtype=np.float32)
    # wst[q*32+ci, kd*9+kh*3+kw, half*64+co] = W[co, ci, kd, kh, kw],
    # replicated over the 4 row groups and the 2 col halves
    wq = W.transpose(1, 2, 3, 4, 0).reshape(CI, 27, CO)
    wq2 = np.concatenate([wq, wq], axis=2)  # duplicate col halves
    wst = np.ascontiguousarray(
        np.broadcast_to(wq2[None], (4, CI, 27, 2 * CO)).reshape(128, 27, 2 * CO)
    ).astype(ml_dtypes.bfloat16)
    bias = np.ascontiguousarray(
        np.concatenate([b, b]).reshape(128, 1).astype(np.float32))
    return wst, bias


def prep_x_slab(xpad, p_lo, n_planes, H=128, W=128):
    """xpad: [CI, D+2, H+2, W+2] zero-padded input. Returns
    [n_planes, 4, CI, H//4+2, W+2] fp32 slab for planes p_lo..p_lo+n_planes."""
    Hq = H // 4
    out = np.empty((n_planes, 4, CI, Hq + 2, W + 2), dtype=np.float32)
    for q in range(4):
        # padded rows 32q .. 32q+34 cover global rows 32q-1 .. 32q+33
        out[:, q] = xpad[:, p_lo:p_lo + n_planes,
                         Hq * q:Hq * q + Hq + 2, :].transpose(1, 0, 2, 3)
    return out


def _prep_inputs(x, W, b):
    x = np.asarray(x, dtype=np.float32)
    wst, bias = prep_weights(W, b)
    xpad = np.pad(x[0], ((0, 0), (1, 1), (1, 1), (1, 1)))
    in_maps = []
    for k in range(N_CORES):
        xs = prep_x_slab(xpad, D_OUT * k, D_IN)
        in_maps.append({"xs": xs, "wst": wst, "bias": bias})
    return in_maps


def run(x, W, b, trace=False):
    """Run the kernel; returns (output, BassKernelResults)."""
    nc = _get_program()
    in_maps = _prep_inputs(x, W, b)
    res = run_bass_kernel_spmd(nc, in_maps, list(range(N_CORES)), trace=trace)
    y = np.concatenate([res.results[k]["y"] for k in range(N_CORES)], axis=1)
    return y[None], res


def kernel(x, W, b):
    y, _ = run(x, W, b)
    return y



# revision 2
# speedup vs baseline: 1.0556x; 1.0556x over previous
"""BinConv3d (sign-binarized 3x3x3 conv, NCDHW) on 8 Trainium2 NeuronCores.

Full inputs in, full output out. Sharding: depth dim D=64 split 8 ways
(8 output planes per core) with a 1-plane halo on the input; conv weights
and bias replicated to every core.

Host prep: each core's input slab is rearranged to [plane, quarter, ci,
34, 130] bf16 — the H dim split into 4 quarter-row panels of 32 rows,
each padded with 1 halo row above/below and 1 zero col left/right, so
the device needs no data reshuffling at all. (bf16 halves the input DMA;
it is sign-preserving, and the matmul consumes bf16 anyway.)

Per-core kernel (Bass/Tile):
  - Two half-panel DMAs per plane (partition = quarter*32 + ci), then
    ScalarE Sign bf16 -> bf16 (zero pads stay zero).
  - Conv = 27 accumulating matmuls (K=32 ci, M=64 co, N=512) per 4-row
    output tile; every (kd, kh, kw) tap is a free-dim AP offset.
  - 16-way PE tiling: quarter q runs on PE row-group q (tile_position
    row 32q); even/odd 4-row blocks run on PE column halves. 8 matmuls
    issue back-to-back per tap and run concurrently: full 128x128 array.
  - PSUM: one 4-bank tile [128, 4, 512] per generation (bank = quarter,
    partitions 0-63 even block / 64-127 odd block), double-buffered.
  - Drain: ONE ScalarE activation (+bias) for the even half and ONE
    VectorE tensor_scalar add (+bias) for the odd half, each [64, 2048]
    across all 4 banks, into separate staging tiles (no false deps).
  - Output DRAM layout is blocked [d, pi, h, co, q, r, w] so each half
    flushes as a single DMA with 8KB-contiguous runs per partition;
    the host transposes back to NCDHW after gather.
"""

import numpy as np
import ml_dtypes

import concourse.bass as bass
import concourse.mybir as mybir
import concourse.tile as tile
from concourse import bacc
from concourse.bass import ts
from concourse.bass_utils import run_bass_kernel_spmd

CI = 32
CO = 64
D_FULL = 64
N_CORES = 8
D_OUT = D_FULL // N_CORES  # output planes per core
D_IN = D_OUT + 2  # input planes per core (1-plane halo each side)

_cache = {}


def build_conv_program(n_in_planes=D_IN, n_out_planes=D_OUT, H=128, W=128,
                       debug=False):
    """Build the per-core Bass program (SPMD: same program on all cores)."""
    f32 = mybir.dt.float32
    bf16 = mybir.dt.bfloat16
    Hq = H // 4          # rows per quarter-panel
    Hqp, Wp = Hq + 2, W + 2
    n_pairs = Hq // 8    # even/odd block pairs per quarter
    assert Hq % 8 == 0 and W == 128

    nc = bacc.Bacc("TRN2", target_bir_lowering=False, debug=debug)
    x_in = nc.declare_dram_parameter(
        "xs", [n_in_planes, 4, CI, Hqp, Wp], bf16, isOutput=False)
    w_in = nc.declare_dram_parameter("wst", [128, 27, 2 * CO], bf16,
                                     isOutput=False)
    b_in = nc.declare_dram_parameter("bias", [128, 1], f32, isOutput=False)
    # blocked output layout: [d, pi, h, co, q, r, w]; host untangles it
    y_out = nc.declare_dram_parameter(
        "y", [n_out_planes, n_pairs, 2, CO, 4, 4, W], f32, isOutput=True)

    with tile.TileContext(nc) as tc:
        with (
            tc.tile_pool(name="const", bufs=1) as constp,
            tc.tile_pool(name="raw", bufs=4) as rawp,
            tc.tile_pool(name="sgn", bufs=10) as sgnp,
            tc.tile_pool(name="stgS", bufs=3) as stgSp,
            tc.tile_pool(name="stgV", bufs=3) as stgVp,
            tc.tile_pool(name="psum", bufs=2, space="PSUM") as psump,
        ):
            wt = constp.tile([128, 27, 2 * CO], bf16)
            nc.sync.dma_start(out=wt[:], in_=w_in[:])
            bs = constp.tile([128, 1], f32)
            nc.sync.dma_start(out=bs[:], in_=b_in[:])

            sgns = {}
            # panel rows are loaded in two halves so the first matmuls can
            # start after half of planes 0-2 arrived: top covers panel rows
            # [0, Hh+2) (blocks 0..Hq/8-1), bottom rows [Hh, Hqp)
            Hh = Hq // 2

            def load_half(p, lo, n):
                raw = rawp.tile([128, n, Wp], bf16, tag="raw")
                nc.sync.dma_start(
                    out=raw[:],
                    in_=x_in[p, :, :, lo:lo + n].rearrange(
                        "q c h w -> (q c) h w"),
                )
                sgn = sgnp.tile([128, n, Wp], bf16, tag="sgn")
                nc.scalar.sign(sgn[:], raw[:])
                return sgn

            def load_plane(p, part):
                if part == 0:
                    sgns[p] = [load_half(p, 0, Hh + 2), None]
                else:
                    sgns[p][1] = load_half(p, Hh, Hh + 2)

            for p in range(3):
                load_plane(p, 0)
            for p in range(3):
                load_plane(p, 1)

            for d in range(n_out_planes):
                if d + 3 < n_in_planes:
                    load_plane(d + 3, 0)
                    load_plane(d + 3, 1)
                for pi in range(n_pairs):
                    # generation: for each quarter, blocks 2*pi (even,
                    # psum partitions 0-63) and 2*pi+1 (odd, 64-127);
                    # bank q holds quarter q's outputs.
                    pt = psump.tile([128, 4, 512], f32, tag="pt",
                                    name=f"pt_{d}_{pi}")
                    for tap in range(27):
                        kd, r = divmod(tap, 9)
                        kh, kw = divmod(r, 3)
                        for q in range(4):
                            for half in range(2):
                                blk = 2 * pi + half
                                top = blk < Hq // 8
                                sg = sgns[d + kd][0 if top else 1]
                                row = 4 * blk + kh - (0 if top else Hh)
                                rhs = sg[32 * q:32 * q + 32,
                                         row:row + 4,
                                         kw:kw + W]
                                nc.tensor.matmul(
                                    pt[64 * half:64 * half + 64, q, :],
                                    lhsT=wt[32 * q:32 * q + 32, tap, 0:CO],
                                    rhs=rhs,
                                    start=(tap == 0),
                                    stop=(tap == 26),
                                    tile_position=(32 * q, 64 * half),
                                    skip_group_check=True,
                                )
                    # drain with bias add: one big op per engine, covering
                    # all 4 banks; separate staging tiles kill false deps
                    stgS = stgSp.tile([128, 4 * 512], f32, tag="stgS")
                    stgV = stgVp.tile([128, 4 * 512], f32, tag="stgV")
                    nc.scalar.activation(
                        stgS[0:64, :],
                        pt[0:64].rearrange("p q n -> p (q n)"),
                        mybir.ActivationFunctionType.Identity,
                        bias=bs[0:64], scale=1.0,
                    )
                    nc.vector.tensor_scalar_add(
                        out=stgV[64:128, :],
                        in0=pt[64:128].rearrange("p q n -> p (q n)"),
                        scalar1=bs[64:128],
                    )
                    # blocked flush: one DMA per col-half, 8KB runs per co
                    nc.sync.dma_start(
                        out=y_out[d, pi, 0],
                        in_=stgS[0:64, :].rearrange(
                            "c (q r w) -> c q r w", q=4, r=4))
                    nc.sync.dma_start(
                        out=y_out[d, pi, 1],
                        in_=stgV[64:128, :].rearrange(
                            "c (q r w) -> c q r w", q=4, r=4))

    nc.compile()
    return nc


def _get_program():
    if "nc" not in _cache:
        _cache["nc"] = build_conv_program()
    return _cache["nc"]


def prep_weights(W, b):
    W = np.asarray(W, dtype=np.float32)
    b = np.asarray(b, dtype=np.float32)
    # wst[q*32+ci, kd*9+kh*3+kw, half*64+co] = W[co, ci, kd, kh, kw],
    # replicated over the 4 row groups and the 2 col halves
    wq = W.transpose(1, 2, 3, 4, 0).reshape(CI, 27, CO)
    wq2 = np.concatenate([wq, wq], axis=2)  # duplicate col halves
    wst = np.ascontiguousarray(
        np.broadcast_to(wq2[None], (4, CI, 27, 2 * CO)).reshape(128, 27, 2 * CO)
    ).astype(ml_dtypes.bfloat16)
    bias = np.ascontiguousarray(
        np.concatenate([b, b]).reshape(128, 1).astype(np.float32))
    return wst, bias


def prep_x_slab(xpad, p_lo, n_planes, H=128, W=128):
    """xpad: [CI, D+2, H+2, W+2] zero-padded bf16 input. Returns
    [n_planes, 4, CI, H//4+2, W+2] bf16 slab for planes p_lo..p_lo+n_planes."""
    Hq = H // 4
    out = np.empty((n_planes, 4, CI, Hq + 2, W + 2), dtype=ml_dtypes.bfloat16)
    for q in range(4):
        # padded rows 32q .. 32q+34 cover global rows 32q-1 .. 32q+33
        out[:, q] = xpad[:, p_lo:p_lo + n_planes,
                         Hq * q:Hq * q + Hq + 2, :].transpose(1, 0, 2, 3)
    return out


def _prep_inputs(x, W, b):
    x = np.asarray(x, dtype=np.float32)
    wst, bias = prep_weights(W, b)
    xb = x[0].astype(ml_dtypes.bfloat16)
    xpad = np.pad(xb, ((0, 0), (1, 1), (1, 1), (1, 1)))
    in_maps = []
    for k in range(N_CORES):
        xs = prep_x_slab(xpad, D_OUT * k, D_IN)
        in_maps.append({"xs": xs, "wst": wst, "bias": bias})
    return in_maps


def _unblock_y(y8, H=128, W=128):
    """[d, pi, h, co, q, r, w] -> [co, d, H, W]; row = 32q + 8pi + 4h + r."""
    y = np.transpose(y8, (3, 0, 4, 1, 2, 5, 6))  # co, d, q, pi, h, r, w
    return np.ascontiguousarray(y).reshape(CO, y8.shape[0], H, W)


def run(x, W, b, trace=False):
    """Run the kernel; returns (output, BassKernelResults)."""
    nc = _get_program()
    in_maps = _prep_inputs(x, W, b)
    res = run_bass_kernel_spmd(nc, in_maps, list(range(N_CORES)), trace=trace)
    y = np.concatenate([_unblock_y(res.results[k]["y"])
                        for k in range(N_CORES)], axis=1)
    return y[None], res


def kernel(x, W, b):
    y, _ = run(x, W, b)
    return y


# revision 7
# speedup vs baseline: 1.1045x; 1.0463x over previous
"""BinConv3d (sign-binarized 3x3x3 conv, NCDHW) on 8 Trainium2 NeuronCores.

Full inputs in, full output out. Sharding: depth dim D=64 split 8 ways
(8 output planes per core) with a 1-plane halo on the input; conv weights
and bias replicated to every core.

Host prep: each core's input slab is rearranged to [plane, quarter, ci,
34, 130] bf16 — the H dim split into 4 quarter-row panels of 32 rows,
each padded with 1 halo row above/below and 1 zero col left/right, so
the device needs no data reshuffling at all. (bf16 halves the input DMA;
it is sign-preserving, and the matmul consumes bf16 anyway.)

Per-core kernel (Bass/Tile):
  - Two half-panel DMAs per plane (partition = quarter*32 + ci) on the
    Scalar HW-DGE queue (input loads never queue behind output flushes),
    launched one plane ahead of their ScalarE Sign bf16 -> bf16 ops.
  - Conv = 27 accumulating matmuls (K=32 ci, M=64 co, N=512) per 4-row
    output tile; every (kd, kh, kw) tap is a free-dim AP offset.
  - 16-way PE tiling: quarter q runs on PE row-group q (tile_position
    row 32q); even/odd 4-row blocks run on PE column halves. 8 matmuls
    issue back-to-back per tap and run concurrently: full 128x128 array.
  - PSUM: per generation, TWO disjoint 2-bank tiles: quarters 0,1 in
    tile A, quarters 2,3 in tile B (partitions 0-63 even block, 64-127
    odd block in each), both double-buffered = all 8 banks.
  - Drain: ScalarE drains ALL of tile A ([128, 1024] + bias), VectorE
    all of tile B — fully independent engines/tiles, 128 lanes each.
  - Output DRAM layout is blocked [d, pi, g, h, co, qq, r, w] so each
    staging tile flushes as one DMA with 4KB-contiguous runs per
    partition; the host transposes back to NCDHW after gather.
"""

import numpy as np
import ml_dtypes

import concourse.bass as bass
import concourse.mybir as mybir
import concourse.tile as tile
from concourse import bacc
from concourse.bass import ts
from concourse.bass_utils import run_bass_kernel_spmd

CI = 32
CO = 64
D_FULL = 64
N_CORES = 8
D_OUT = D_FULL // N_CORES  # output planes per core
D_IN = D_OUT + 2  # input planes per core (1-plane halo each side)

_cache = {}


def build_conv_program(n_in_planes=D_IN, n_out_planes=D_OUT, H=128, W=128,
                       debug=False):
    """Build the per-core Bass program (SPMD: same program on all cores)."""
    f32 = mybir.dt.float32
    bf16 = mybir.dt.bfloat16
    Hq = H // 4          # rows per quarter-panel
    Hqp, Wp = Hq + 2, W + 2
    n_pairs = Hq // 8    # even/odd block pairs per quarter
    assert Hq % 8 == 0 and W == 128

    nc = bacc.Bacc("TRN2", target_bir_lowering=False, debug=debug)
    x_in = nc.declare_dram_parameter(
        "xs", [n_in_planes, 4, CI, Hqp, Wp], bf16, isOutput=False)
    w_in = nc.declare_dram_parameter("wst", [128, 27, 2 * CO], bf16,
                                     isOutput=False)
    b_in = nc.declare_dram_parameter("bias", [128, 1], f32, isOutput=False)
    # blocked output layout: [d, pi, g, (h co), (qq r w)]; host untangles.
    # row = 32*(2g+qq) + 8pi + 4h + r
    y_out = nc.declare_dram_parameter(
        "y", [n_out_planes, n_pairs, 2, 128, 2 * 4 * W], f32, isOutput=True)

    with tile.TileContext(nc) as tc:
        with (
            tc.tile_pool(name="const", bufs=1) as constp,
            tc.tile_pool(name="raw", bufs=8) as rawp,
            tc.tile_pool(name="sgn", bufs=10) as sgnp,
            tc.tile_pool(name="stgA", bufs=3) as stgAp,
            tc.tile_pool(name="stgB", bufs=3) as stgBp,
            tc.tile_pool(name="psumA", bufs=2, space="PSUM") as psumAp,
            tc.tile_pool(name="psumB", bufs=2, space="PSUM") as psumBp,
        ):
            wt = constp.tile([128, 27, 2 * CO], bf16)
            nc.sync.dma_start(out=wt[:], in_=w_in[:])
            bs = constp.tile([128, 1], f32)
            nc.sync.dma_start(out=bs[:], in_=b_in[:])

            raws = {}
            sgns = {}
            # panel rows are loaded in two halves so the first matmuls can
            # start after half of planes 0-2 arrived: top covers panel rows
            # [0, Hh+2) (blocks 0..Hq/8-1), bottom rows [Hh, Hqp)
            Hh = Hq // 2

            def launch_half(p, part, eng):
                lo = 0 if part == 0 else Hh
                n = Hh + 2
                raw = rawp.tile([128, n, Wp], bf16, tag="raw")
                eng.dma_start(
                    out=raw[:],
                    in_=x_in[p, :, :, lo:lo + n].rearrange(
                        "q c h w -> (q c) h w"),
                )
                raws.setdefault(p, [None, None])[part] = raw

            def sign_half(p, part):
                raw = raws[p][part]
                sgn = sgnp.tile(list(raw.shape), bf16, tag="sgn")
                nc.scalar.sign(sgn[:], raw[:])
                sgns.setdefault(p, [None, None])[part] = sgn

            # head: tops on the scalar queue, bottoms on sync (parallel)
            for p in range(3):
                launch_half(p, 0, nc.scalar)
                launch_half(p, 1, nc.sync)
            for p in range(3):
                sign_half(p, 0)
            for p in range(3):
                sign_half(p, 1)

            for d in range(n_out_planes):
                prefetch = d + 3 < n_in_planes
                if prefetch:
                    launch_half(d + 3, 0, nc.scalar)
                    launch_half(d + 3, 1, nc.scalar)
                for pi in range(n_pairs):
                    # generation: for each quarter, blocks 2*pi (even,
                    # psum partitions 0-63) and 2*pi+1 (odd, 64-127);
                    # quarters 0,1 -> tile A, quarters 2,3 -> tile B.
                    ptA = psumAp.tile([128, 2, 512], f32, tag="ptA",
                                      name=f"ptA_{d}_{pi}")
                    ptB = psumBp.tile([128, 2, 512], f32, tag="ptB",
                                      name=f"ptB_{d}_{pi}")
                    for tap in range(27):
                        kd, r = divmod(tap, 9)
                        kh, kw = divmod(r, 3)
                        for q in range(4):
                            pt = ptA if q < 2 else ptB
                            for half in range(2):
                                blk = 2 * pi + half
                                top = blk < Hq // 8
                                sg = sgns[d + kd][0 if top else 1]
                                row = 4 * blk + kh - (0 if top else Hh)
                                rhs = sg[32 * q:32 * q + 32,
                                         row:row + 4,
                                         kw:kw + W]
                                nc.tensor.matmul(
                                    pt[64 * half:64 * half + 64, q % 2, :],
                                    lhsT=wt[32 * q:32 * q + 32, tap, 0:CO],
                                    rhs=rhs,
                                    start=(tap == 0),
                                    stop=(tap == 26),
                                    tile_position=(32 * q, 64 * half),
                                    skip_group_check=True,
                                )
                    # drain with bias add: ScalarE takes all of tile A,
                    # VectorE all of tile B — no shared tiles, 128 lanes
                    stgA = stgAp.tile([128, 2 * 512], f32, tag="stgA")
                    stgB = stgBp.tile([128, 2 * 512], f32, tag="stgB")
                    nc.vector.tensor_scalar_add(
                        out=stgB[:],
                        in0=ptB[:].rearrange("p q n -> p (q n)"),
                        scalar1=bs[:],
                    )
                    nc.scalar.activation(
                        stgA[:],
                        ptA[:].rearrange("p q n -> p (q n)"),
                        mybir.ActivationFunctionType.Identity,
                        bias=bs[:], scale=1.0,
                    )
                    # blocked flush: one fully-contiguous DMA per pair
                    for g, stg in ((0, stgA), (1, stgB)):
                        nc.sync.dma_start(out=y_out[d, pi, g], in_=stg[:])
                    # interleave next plane's signs between drains so the
                    # scalar FIFO never blocks a drain on a pending DMA
                    if prefetch and pi < 2:
                        sign_half(d + 3, pi)

    nc.compile()
    return nc


def _get_program():
    if "nc" not in _cache:
        _cache["nc"] = build_conv_program()
    return _cache["nc"]


def prep_weights(W, b):
    W = np.asarray(W, dtype=np.float32)
    b = np.asarray(b, dtype=np.float32)
    # wst[q*32+ci, kd*9+kh*3+kw, half*64+co] = W[co, ci, kd, kh, kw],
    # replicated over the 4 row groups and the 2 col halves
    wq = W.transpose(1, 2, 3, 4, 0).reshape(CI, 27, CO)
    wq2 = np.concatenate([wq, wq], axis=2)  # duplicate col halves
    wst = np.ascontiguousarray(
        np.broadcast_to(wq2[None], (4, CI, 27, 2 * CO)).reshape(128, 27, 2 * CO)
    ).astype(ml_dtypes.bfloat16)
    bias = np.ascontiguousarray(
        np.concatenate([b, b]).reshape(128, 1).astype(np.float32))
    return wst, bias


def prep_x_slab(xpad, p_lo, n_planes, H=128, W=128):
    """xpad: [CI, D+2, H+2, W+2] zero-padded bf16 input. Returns
    [n_planes, 4, CI, H//4+2, W+2] bf16 slab for planes p_lo..p_lo+n_planes."""
    Hq = H // 4
    out = np.empty((n_planes, 4, CI, Hq + 2, W + 2), dtype=ml_dtypes.bfloat16)
    for q in range(4):
        # padded rows 32q .. 32q+34 cover global rows 32q-1 .. 32q+33
        out[:, q] = xpad[:, p_lo:p_lo + n_planes,
                         Hq * q:Hq * q + Hq + 2, :].transpose(1, 0, 2, 3)
    return out


def _prep_inputs(x, W, b):
    x = np.asarray(x, dtype=np.float32)
    wst, bias = prep_weights(W, b)
    xb = x[0].astype(ml_dtypes.bfloat16)
    xpad = np.pad(xb, ((0, 0), (1, 1), (1, 1), (1, 1)))
    in_maps = []
    for k in range(N_CORES):
        xs = prep_x_slab(xpad, D_OUT * k, D_IN)
        in_maps.append({"xs": xs, "wst": wst, "bias": bias})
    return in_maps


def _unblock_y(y8, H=128, W=128):
    """[d, pi, g, (h co), (qq r w)] -> [co, d, H, W];
    row = 32*(2g+qq) + 8pi + 4h + r."""
    nd = y8.shape[0]
    y = y8.reshape(nd, 4, 2, 2, CO, 2, 4, W)  # d,pi,g,h,co,qq,r,w
    y = np.transpose(y, (4, 0, 2, 5, 1, 3, 6, 7))  # co,d,g,qq,pi,h,r,w
    return np.ascontiguousarray(y).reshape(CO, nd, H, W)


def run(x, W, b, trace=False):
    """Run the kernel; returns (output, BassKernelResults)."""
    nc = _get_program()
    in_maps = _prep_inputs(x, W, b)
    res = run_bass_kernel_spmd(nc, in_maps, list(range(N_CORES)), trace=trace)
    y = np.concatenate([_unblock_y(res.results[k]["y"])
                        for k in range(N_CORES)], axis=1)
    return y[None], res


def kernel(x, W, b):
    y, _ = run(x, W, b)
    return y
